# revision 2
# baseline (speedup 1.0000x reference)
"""Causal multi-head attention (B=4, S=2048, D=1024, H=16) on 8 TRN2 cores.

Sharding: core c -> (batch b = c//2, head-group g = c%2, 8 heads each).
Host pre-transposes/splits inputs; device returns per-core partial outputs
y_c = attn_heads(g) @ wo[g-rows]; host sums the two partials per batch.

v2: fully software-pipelined single-phase schedule. Projections, attention
and the output projection are interleaved per 512-position m-slab so the
tensor engine never idles long enough to drop the HAM clock, input DMA
streams just-in-time on the ACT HWDGE ring, P^T transposes on the sync
ring, scatters + y-writes on gpsimd SWDGE.

Precision: scores must be near-fp32 (softmax at scale ~1000 is argmax-like).
All score-path matmuls run in fp16 hi/lo splits (fp16 products are exact in
fp32 PSUM accumulation):
  - q/k projections: 3 passes  (xhi@whi + xlo@whi + xhi@wlo)    -> exact qh
  - qk^T: 2 passes with K=128 packing  [qhi;qhi].[khi;klo] + [qlo;qlo].[khi;klo]
The 1/sqrt(dk)=0.125 score scale is folded into wq on the host.
Value path (V, P, wo) in plain fp16.
"""

from collections import deque

import numpy as np

import concourse.bacc as bacc
import concourse.tile as tile
from concourse import mybir
from concourse.bass_utils import run_bass_kernel_spmd

B, S, D = 4, 2048, 1024
H, DK = 16, 64
HL = 8            # heads per core
DL = HL * DK      # 512 local channels
N_CORES = 8
P = 128           # partitions
MT = S // 512     # 4 m-slabs of 512
NT = DL // P      # 4 channel slabs of 128
KT = D // P       # 8 contraction tiles
QT = S // P       # 16 q tiles
CHUNK = 1024      # score chunk (2 PSUM banks)
LAG = 3           # alpha->beta software pipeline depth

f32 = mybir.dt.float32
f16 = mybir.dt.float16
AX = mybir.AxisListType.X
ALU = mybir.AluOpType
AF = mybir.ActivationFunctionType

_cache = {}


def _build():
    nc = bacc.Bacc("TRN2", target_bir_lowering=False)

    def din(name, shape, dt=f16):
        return nc.dram_tensor(name, shape, dt, kind="ExternalInput").ap()

    xq_hi = din("xq_hi", [D, S]); xq_lo = din("xq_lo", [D, S])
    xk_hi = din("xk_hi", [D, S]); xk_lo = din("xk_lo", [D, S])
    xv16 = din("xv16", [D, S])
    wq_hi = din("wq_hi", [D, DL]); wq_lo = din("wq_lo", [D, DL])
    wk_hi = din("wk_hi", [D, DL]); wk_lo = din("wk_lo", [D, DL])
    wv16 = din("wv16", [D, DL])
    wo16 = din("wo16", [DL, D])
    maskc = din("maskc", [P, P], f32)
    y = nc.dram_tensor("y", [S, D], f32, kind="ExternalOutput").ap()

    with tile.TileContext(nc) as tc:
        _body(nc, tc,
              xq_hi, xq_lo, xk_hi, xk_lo, xv16,
              wq_hi, wq_lo, wk_hi, wk_lo, wv16, wo16, maskc, y)
    nc.compile()
    return nc


def _body(nc, tc, xq_hi, xq_lo, xk_hi, xk_lo, xv16,
          wq_hi, wq_lo, wk_hi, wk_lo, wv16, wo16, maskc, y):
    from contextlib import ExitStack
    ctx = ExitStack()
    with ctx:
        # ---------- pools ----------
        persist = ctx.enter_context(tc.tile_pool(name="persist", bufs=1))
        qring = ctx.enter_context(tc.tile_pool(name="qring", bufs=1))
        xpool = ctx.enter_context(tc.tile_pool(name="xpool", bufs=1))
        stage = ctx.enter_context(tc.tile_pool(name="stage", bufs=4))
        ppool = ctx.enter_context(tc.tile_pool(name="ppool", bufs=4))
        ptpool = ctx.enter_context(tc.tile_pool(name="ptpool", bufs=4))
        stat = ctx.enter_context(tc.tile_pool(name="stat", bufs=4))
        ostage = ctx.enter_context(tc.tile_pool(name="ostage", bufs=2))
        opool = ctx.enter_context(tc.tile_pool(name="opool", bufs=1))
        otp = ctx.enter_context(tc.tile_pool(name="otp", bufs=2))
        ypool = ctx.enter_context(tc.tile_pool(name="ypool", bufs=2))
        gen_ps = ctx.enter_context(tc.tile_pool(name="genps", bufs=2, space="PSUM"))
        scpool = ctx.enter_context(tc.tile_pool(name="scps", bufs=2, space="PSUM"))
        pvpool = ctx.enter_context(tc.tile_pool(name="pvps", bufs=2, space="PSUM"))

        # ---------- persistent tiles ----------
        ktx = [persist.tile([P, S], f16, tag=f"ktx_{h}", name=f"ktx_{h}")
               for h in range(HL)]
        vsb = [persist.tile([P, DL], f16, tag=f"v_{m}", name=f"v_{m}")
               for m in range(QT)]
        mask_sb = persist.tile([P, P], f32, tag="mask")
        wq_h = persist.tile([P, KT, DL], f16, tag="wqh", name="wq_h")
        wq_l = persist.tile([P, KT, DL], f16, tag="wql", name="wq_l")
        wk_h = persist.tile([P, KT, DL], f16, tag="wkh", name="wk_h")
        wk_l = persist.tile([P, KT, DL], f16, tag="wkl", name="wk_l")
        wv_sb = persist.tile([P, KT, DL], f16, tag="wv", name="wv_sb")
        wo_sb = persist.tile([P, NT, D], f16, tag="wo", name="wo_sb")

        nc.sync.dma_start(out=mask_sb, in_=maskc)
        for t, dr in ((wq_h, wq_hi), (wq_l, wq_lo), (wk_h, wk_hi),
                      (wk_l, wk_lo), (wv_sb, wv16)):
            nc.scalar.dma_start(out=t, in_=dr.rearrange("(k p) n -> p k n", p=P))
        nc.scalar.dma_start(out=wo_sb, in_=wo16.rearrange("(j p) n -> p j n", p=P))

        # ---------- input streaming (ACT HWDGE ring) ----------
        def load_x_slab(m, which):
            """16 per-ktile [P,512] fp16 tiles (hi+lo) for q or k."""
            hi_d, lo_d = (xq_hi, xq_lo) if which == "q" else (xk_hi, xk_lo)
            hi_r = hi_d.rearrange("(k p) s -> p k s", p=P)
            lo_r = lo_d.rearrange("(k p) s -> p k s", p=P)
            msl = slice(m * 512, (m + 1) * 512)
            tiles = []
            for kt in range(KT):
                th = xpool.tile([P, 512], f16, tag="xs", bufs=24,
                                name=f"x{which}h{m}_{kt}")
                nc.scalar.dma_start(out=th, in_=hi_r[:, kt, msl])
                tl = xpool.tile([P, 512], f16, tag="xs", bufs=24,
                                name=f"x{which}l{m}_{kt}")
                nc.scalar.dma_start(out=tl, in_=lo_r[:, kt, msl])
                tiles.append((th, tl))
            return tiles

        def load_xv_slab(m):
            xv_r = xv16.rearrange("(k p) s -> p k s", p=P)
            msl = slice(m * 512, (m + 1) * 512)
            tiles = []
            for kt in range(KT):
                t = xpool.tile([P, 512], f16, tag="xv", bufs=10,
                               name=f"xv{m}_{kt}")
                nc.scalar.dma_start(out=t, in_=xv_r[:, kt, msl])
                tiles.append(t)
            return tiles

        # ---------- projections ----------
        cur_q = {}

        def new_qring(m):
            for h in range(HL):
                qh_t = qring.tile([P, 512], f16, tag=f"qh{h}", name=f"qh{h}_{m}")
                ql_t = qring.tile([P, 512], f16, tag=f"ql{h}", name=f"ql{h}_{m}")
                cur_q[h] = (qh_t, ql_t)

        def emit_vproj(m, xv_tiles):
            for blk in range(4):
                ps = gen_ps.tile([P, DL], f32, tag="gen", name="vps")
                for kt in range(KT):
                    nc.tensor.matmul(ps[:], xv_tiles[kt][:, blk * P:(blk + 1) * P],
                                     wv_sb[:, kt],
                                     start=(kt == 0), stop=(kt == KT - 1))
                nc.scalar.copy(vsb[4 * m + blk][:], ps[:])

        def emit_qkproj(m, which, tiles):
            whi, wlo = (wq_h, wq_l) if which == "q" else (wk_h, wk_l)
            msl = slice(m * 512, (m + 1) * 512)
            for n in range(NT):
                ps = gen_ps.tile([P, 512], f32, tag="gen", name="qkps")
                last = 3 * KT - 1
                i = 0
                for kt in range(KT):
                    th, tl = tiles[kt]
                    for lhsT, rhs in (
                        (whi[:, kt, n * P:(n + 1) * P], th),
                        (whi[:, kt, n * P:(n + 1) * P], tl),
                        (wlo[:, kt, n * P:(n + 1) * P], th),
                    ):
                        nc.tensor.matmul(ps[:], lhsT, rhs,
                                         start=(i == 0), stop=(i == last))
                        i += 1
                st_hi = stage.tile([P, 512], f16, tag="st_hi")
                st_lo = stage.tile([P, 512], f16, tag="st_lo")
                nc.vector.tensor_copy(st_hi[:], ps[:])
                nc.vector.tensor_tensor(out=st_lo, in0=ps[:], in1=st_hi,
                                        op=ALU.subtract)
                for hh in range(2):
                    h = 2 * n + hh
                    rsl = slice(hh * DK, hh * DK + DK)
                    if which == "q":
                        qh_t, ql_t = cur_q[h]
                        nc.gpsimd.dma_start(out=qh_t[0:DK, :], in_=st_hi[rsl, :])
                        nc.gpsimd.dma_start(out=qh_t[DK:P, :], in_=st_hi[rsl, :])
                        nc.gpsimd.dma_start(out=ql_t[0:DK, :], in_=st_lo[rsl, :])
                        nc.gpsimd.dma_start(out=ql_t[DK:P, :], in_=st_lo[rsl, :])
                    else:
                        nc.gpsimd.dma_start(out=ktx[h][0:DK, msl], in_=st_hi[rsl, :])
                        nc.gpsimd.dma_start(out=ktx[h][DK:P, msl], in_=st_lo[rsl, :])

        # ---------- attention ----------
        state = {}
        sched = deque()

        def get_qt_state(qt):
            if qt not in state:
                klen = (qt + 1) * P
                nch = 1 if klen <= CHUNK else 2
                state[qt] = dict(
                    m_t=stat.tile([P, 2 * HL], f32, tag="m1", name="m1t"),
                    z_t=stat.tile([P, 2 * HL], f32, tag="z1", name="z1t"),
                    ostg=ostage.tile([P, DL], f16, tag="ostg", name="ostg"),
                    nch=nch, pc={}, pt={}, osb={},
                )
            return state[qt]

        def chunks_of(qt):
            klen = (qt + 1) * P
            return [(0, klen)] if klen <= CHUNK else [(0, CHUNK), (CHUNK, klen)]

        def alpha(qt, h):
            st = get_qt_state(qt)
            klen = (qt + 1) * P
            qsl = slice((qt % 4) * P, (qt % 4 + 1) * P)
            lq = cur_q[h][0][:, qsl]
            ll = cur_q[h][1][:, qsl]
            pc = ppool.tile([P, klen], f16, tag="p", padded_shape=[P, S], name="pc")
            st["pc"][h] = pc
            for ci, (c0, c1) in enumerate(chunks_of(qt)):
                cl = c1 - c0
                sc = scpool.tile([P, CHUNK], f32, tag="scores", name="sc")
                ngs = [(ng * 512, min(512, cl - ng * 512))
                       for ng in range((cl + 511) // 512)]
                for n0, nn in ngs:
                    rk = ktx[h][:, c0 + n0:c0 + n0 + nn]
                    nc.tensor.matmul(sc[:, n0:n0 + nn], lq, rk, start=True, stop=False)
                for n0, nn in ngs:
                    rk = ktx[h][:, c0 + n0:c0 + n0 + nn]
                    nc.tensor.matmul(sc[:, n0:n0 + nn], ll, rk, start=False, stop=True)
                if c1 == klen:  # diagonal block: causal mask
                    nc.vector.tensor_tensor(
                        out=sc[:, cl - P:cl], in0=sc[:, cl - P:cl],
                        in1=mask_sb[:], op=ALU.add)
                mt = st["m_t"][:, 2 * h + ci:2 * h + ci + 1]
                zt = st["z_t"][:, 2 * h + ci:2 * h + ci + 1]
                nc.vector.reduce_max(mt, sc[:, :cl], axis=AX, negate=True)
                nc.scalar.activation(pc[:, c0:c1], sc[:, :cl], AF.Exp,
                                     bias=mt, accum_out=zt)
            pt = ptpool.tile([P, QT, P], f16, tag="pt", name="pt")
            st["pt"][h] = pt
            nc.sync.dma_start_transpose(pt[:, 0:klen // P, :], pc[:])

        def beta(qt, h):
            st = state[qt]
            pt = st["pt"][h]
            for ci, (c0, c1) in enumerate(chunks_of(qt)):
                nkb = (c1 - c0) // P
                ops = pvpool.tile([P, DK], f32, tag="pv", name="pvt")
                for kb in range(nkb):
                    nc.tensor.matmul(
                        ops[:], pt[:, c0 // P + kb, :],
                        vsb[c0 // P + kb][:, h * DK:(h + 1) * DK],
                        start=(kb == 0), stop=(kb == nkb - 1))
                if st["nch"] == 1:
                    rh = stat.tile([P, 1], f32, tag="rh")
                    nc.vector.reciprocal(rh, st["z_t"][:, 2 * h:2 * h + 1])
                    nc.scalar.activation(
                        st["ostg"][:, h * DK:(h + 1) * DK], ops[:], AF.Copy, scale=rh)
                else:
                    osb = opool.tile([P, DK], f32, tag=f"o{ci}_{h}", name=f"osb{ci}_{h}")
                    nc.scalar.copy(osb[:], ops[:])
                    st["osb"][(h, ci)] = osb

        def finish_and_project(qt):
            st = state[qt]
            ostg = st["ostg"]
            if st["nch"] == 2:
                m_t, z_t = st["m_t"], st["z_t"]
                ev = slice(0, 2 * HL, 2)
                od = slice(1, 2 * HL, 2)
                m1, m2 = m_t[:, ev], m_t[:, od]   # negated chunk maxes
                z1, z2 = z_t[:, ev], z_t[:, od]
                negM = stat.tile([P, HL], f32, tag="negM")
                nc.vector.tensor_tensor(out=negM, in0=m1, in1=m2, op=ALU.min)
                d1 = stat.tile([P, HL], f32, tag="d1")
                d2 = stat.tile([P, HL], f32, tag="d2")
                nc.vector.tensor_tensor(out=d1, in0=negM, in1=m1, op=ALU.subtract)
                nc.vector.tensor_tensor(out=d2, in0=negM, in1=m2, op=ALU.subtract)
                w1 = stat.tile([P, HL], f32, tag="w1")
                w2 = stat.tile([P, HL], f32, tag="w2")
                nc.scalar.activation(w1, d1, AF.Exp)
                nc.scalar.activation(w2, d2, AF.Exp)
                zz = stat.tile([P, HL], f32, tag="zz")
                zs = stat.tile([P, HL], f32, tag="zs")
                nc.vector.tensor_tensor(out=zz, in0=w1, in1=z1, op=ALU.mult)
                nc.vector.tensor_tensor(out=zs, in0=w2, in1=z2, op=ALU.mult)
                ztot = stat.tile([P, HL], f32, tag="ztot")
                nc.vector.tensor_tensor(out=ztot, in0=zz, in1=zs, op=ALU.add)
                r_t = stat.tile([P, HL], f32, tag="r")
                nc.vector.reciprocal(r_t, ztot)
                s1 = stat.tile([P, HL], f32, tag="s1")
                s2 = stat.tile([P, HL], f32, tag="s2")
                nc.vector.tensor_tensor(out=s1, in0=w1, in1=r_t, op=ALU.mult)
                nc.vector.tensor_tensor(out=s2, in0=w2, in1=r_t, op=ALU.mult)
                for h in range(HL):
                    osl = ostg[:, h * DK:(h + 1) * DK]
                    nc.scalar.activation(osl, st["osb"][(h, 0)][:], AF.Copy,
                                         scale=s1[:, h:h + 1])
                    nc.vector.scalar_tensor_tensor(
                        out=osl, in0=st["osb"][(h, 1)][:], scalar=s2[:, h:h + 1],
                        in1=osl, op0=ALU.mult, op1=ALU.add)
            # fused output projection for this q-tile
            oT = otp.tile([P, NT, P], f16, tag="oT", name="oT")
            nc.sync.dma_start_transpose(oT, ostg[:])
            for half in range(2):
                ps = gen_ps.tile([P, 512], f32, tag="gen", name="yps")
                for j in range(NT):
                    nc.tensor.matmul(
                        ps[:], oT[:, j, :],
                        wo_sb[:, j, half * 512:(half + 1) * 512],
                        start=(j == 0), stop=(j == NT - 1))
                ysb = ypool.tile([P, 512], f32, tag="y", name="ysb")
                nc.scalar.copy(ysb[:], ps[:])
                nc.gpsimd.dma_start(
                    out=y[qt * P:(qt + 1) * P, half * 512:(half + 1) * 512],
                    in_=ysb[:])
            del state[qt]

        def pop_one():
            bqt, bh = sched.popleft()
            beta(bqt, bh)
            if bh == HL - 1:
                finish_and_project(bqt)

        def emit_attention_group(g):
            for qt in range(4 * g, 4 * g + 4):
                for h in range(HL):
                    alpha(qt, h)
                    sched.append((qt, h))
                    if len(sched) > LAG:
                        pop_one()

        # ---------- fused main schedule ----------
        xq_tiles = load_x_slab(0, "q")
        xk_tiles = load_x_slab(0, "k")
        xv_tiles = load_xv_slab(0)
        for m in range(MT):
            new_qring(m)
            emit_qkproj(m, "q", xq_tiles)
            if m + 1 < MT:
                xq_tiles = load_x_slab(m + 1, "q")
            emit_qkproj(m, "k", xk_tiles)
            if m + 1 < MT:
                xk_tiles = load_x_slab(m + 1, "k")
            emit_vproj(m, xv_tiles)
            if m + 1 < MT:
                xv_tiles = load_xv_slab(m + 1)
            emit_attention_group(m)
        while sched:
            pop_one()


def _host_prep(q, k, v, wq, wk, wv, wo):
    """Build the 8 per-core input maps."""
    def split16(x):
        hi = x.astype(np.float16)
        lo = (x - hi.astype(np.float32)).astype(np.float16)
        return hi, lo

    mask = np.triu(np.full((P, P), -1e30, np.float32), k=1)
    in_maps = []
    per_b = {}
    for b in range(B):
        xqT = np.ascontiguousarray(q[b].T.astype(np.float32))
        xkT = np.ascontiguousarray(k[b].T.astype(np.float32))
        xvT = np.ascontiguousarray(v[b].T.astype(np.float32))
        qhi, qlo = split16(xqT)
        khi, klo = split16(xkT)
        per_b[b] = (qhi, qlo, khi, klo, xvT.astype(np.float16))
    per_g = {}
    for g in range(2):
        cs = slice(g * DL, (g + 1) * DL)
        # fold the 1/sqrt(dk) score scale into wq
        wqc = np.ascontiguousarray(wq[:, cs].astype(np.float32)) * 0.125
        wkc = np.ascontiguousarray(wk[:, cs].astype(np.float32))
        wq_h, wq_l = split16(wqc)
        wk_h, wk_l = split16(wkc)
        per_g[g] = (wq_h, wq_l, wk_h, wk_l,
                    np.ascontiguousarray(wv[:, cs]).astype(np.float16),
                    np.ascontiguousarray(wo[cs, :]).astype(np.float16))
    for c in range(N_CORES):
        b, g = c // 2, c % 2
        qhi, qlo, khi, klo, xv = per_b[b]
        wq_h, wq_l, wk_h, wk_l, wv_c, wo_c = per_g[g]
        in_maps.append({
            "xq_hi": qhi, "xq_lo": qlo, "xk_hi": khi, "xk_lo": klo,
            "xv16": xv, "wq_hi": wq_h, "wq_lo": wq_l,
            "wk_hi": wk_h, "wk_lo": wk_l, "wv16": wv_c, "wo16": wo_c,
            "maskc": mask,
        })
    return in_maps


def kernel(q, k, v, wq, wk, wv, wo):
    if "nc" not in _cache:
        _cache["nc"] = _build()
    nc = _cache["nc"]
    in_maps = _host_prep(np.asarray(q), np.asarray(k), np.asarray(v),
                         np.asarray(wq), np.asarray(wk), np.asarray(wv),
                         np.asarray(wo))
    res = run_bass_kernel_spmd(nc, in_maps, list(range(N_CORES)))
    out = np.empty((B, S, D), np.float32)
    for b in range(B):
        out[b] = res.results[2 * b]["y"] + res.results[2 * b + 1]["y"]
    return out


if __name__ == "__main__":
    d = np.load("/root/problem/inputs_cache.npz")
    out = kernel(d["q"], d["k"], d["v"], d["wq"], d["wk"], d["wv"], d["wo"])
    ref = d["ref"]
    rel = np.linalg.norm(out - ref) / np.linalg.norm(ref)
    print(f"Relative error: {rel:.4e}")
